# revision 1
# baseline (speedup 1.0000x reference)
"""Trainium2 Bass kernel for CausalModulatedAttention (transposed form).

Full-input contract: kernel(**inputs) takes the unsharded numpy inputs and
returns the full (B, L, D) float32 output.

Sharding: core = 2*b + g (b = batch, g = head-group of 8 heads).  Everything
is computed in j-transposed orientation so softmax-ready tiles feed attn@v
directly as matmul moving operands (no PE transposes / PSUM round trips):

  - scores^T[j, i] = k_j . q_i on PE (j chunked into 4 blocks of 128);
    exp(scores) on ACT runs independently of the causal-graph bias.
  - the pairwise causal graph (shared by all heads) is rank-split in
    interleaved 4-row j-groups (j = 8k + 4g + u) so both cores of a pair do
    identical-shape, balanced causal work: gelu(hc_i + he_j + b1) over the
    causal suffix, reduced over the hidden dim by sparse 32-col stationary
    matmuls, then tanh.  Halves are exchanged as fp8 through one per-pair
    AllGather and re-interleaved by two constant fp8 permutation matmuls per
    j-chunk; the bias enters multiplicatively as e2 = exp(qk) * expG with
    expG = exp(alpha/2*tanh + trimask) (mask -> exp = 0), computed on
    DVE/GpSimd after the exchange lands, off the ACT queue.
  - softmax row sums (over j = partitions) come from ones-column matmuls
    into one shared PSUM accumulation group (all heads in partitions 0..7);
    normalization is folded into the PSUM->SBUF copy via a broadcast
    inverse tile built by a rank-8 selector matmul.
  - attn@v: otp[hd, i] accumulates over j-chunks with suffix extents.
  - each core emits a partial output (its heads' half of the d-contraction
    in the final projection); the host adds the two halves.

All matmul operands bf16 (exchange/permute fp8), fp32 PSUM accumulation.
"""

import math

import numpy as np
import ml_dtypes

import concourse.bass as bass
import concourse.mybir as mybir
import concourse.tile as tile
from concourse import bacc
from concourse.bass_utils import run_bass_kernel_spmd

BF = mybir.dt.bfloat16
F8 = mybir.dt.float8e4
F32 = mybir.dt.float32
AF = mybir.ActivationFunctionType
ALU = mybir.AluOpType

B, L, D = 4, 512, 1024
H, HD, CD = 16, 64, 32
ALPHA = 0.3
N_CORES = 8
HPC = 8               # heads per core
DPC = HPC * HD        # 512 d-columns per core
NEG = -1.0e30
AW, BW = 512, 256     # i-extents of pairwise blocks A (k 0..31) and B (32..63)
XW = AW + BW          # exchanged columns (A then B)
# cpk packing: w2t (1024) | trimask (512) | ones8 (64) | sel8 (512);
# p8 carries P0|P1 as fp8 for the gx permutes
CPK = 1024 + 512 + 64 + 512


def _bf(a):
    return np.ascontiguousarray(a.astype(ml_dtypes.bfloat16))


def _f32(a):
    return np.ascontiguousarray(a.astype(np.float32))


def build_program():
    nc = bacc.Bacc("TRN2", num_devices=N_CORES, target_bir_lowering=False,
                   debug=False)

    boot_d = nc.dram_tensor("boot", [128, 1280], BF, kind="ExternalInput")
    xta_d = nc.dram_tensor("xta", [128, 8 * L], BF, kind="ExternalInput")
    xje_d = nc.dram_tensor("xje", [128, 8 * 256], BF, kind="ExternalInput")
    cpk_d = nc.dram_tensor("cpk", [128, CPK], BF, kind="ExternalInput")
    p8_d = nc.dram_tensor("p8", [128, 256], F8, kind="ExternalInput")
    wk_d = nc.dram_tensor("wka", [128, 8 * DPC], BF, kind="ExternalInput")
    wq_d = nc.dram_tensor("wqa", [128, 8 * DPC], BF, kind="ExternalInput")
    wv_d = nc.dram_tensor("wva", [128, 8 * DPC], BF, kind="ExternalInput")
    wo_d = nc.dram_tensor("woa", [128, 4 * D], BF, kind="ExternalInput")
    b1_d = nc.dram_tensor("b1x4", [128, 1], F32, kind="ExternalInput")
    b2_d = nc.dram_tensor("b2h", [128, 1], F32, kind="ExternalInput")
    out_d = nc.dram_tensor("out", [L, D], BF, kind="ExternalOutput")

    with tile.TileContext(nc) as tc:
        with (
            tc.tile_pool(name="consts", bufs=1) as consts,
            tc.tile_pool(name="work", bufs=2) as work,
            tc.tile_pool(name="etp", bufs=8) as etp,
            tc.tile_pool(name="dram", bufs=1, space="DRAM") as dpool,
            tc.tile_pool(name="pps", bufs=2, space="PSUM") as pps,
        ):
            def load(name, shape, dt, src):
                t = consts.tile(shape, dt, tag=name)
                nc.sync.dma_start(out=t[:], in_=src)
                return t

            # dsb/di + xje + warmup on the Pool queue, ahead of everything
            di = dpool.tile([128, 16], BF, tag="di")
            do = dpool.tile([2, 128, 16], BF, tag="do")
            dsb = consts.tile([128, 16], BF, tag="dsb")
            nc.gpsimd.memset(dsb[:], 0.0)
            nc.gpsimd.dma_start(out=di[:], in_=dsb[:])
            bootp = consts.tile([128, 1280], BF, tag="bootp")
            nc.gpsimd.dma_start(out=bootp[:], in_=boot_d[:, :])
            xje = consts.tile([128, 8 * 256], BF, tag="xje")
            nc.gpsimd.dma_start(out=xje[:], in_=xje_d[:, :])
            cpk = consts.tile([128, CPK], BF, tag="cpk")
            nc.gpsimd.dma_start(out=cpk[:], in_=cpk_d[:, :])
            nc.gpsimd.collective_compute(
                "AllGather", ALU.bypass,
                replica_groups=[[0, 1], [2, 3], [4, 5], [6, 7]],
                ins=[di[:, :].opt()], outs=[do[:, :, :].opt()])

            boot = bootp
            xta = consts.tile([128, 8 * L], BF, tag="xta")
            nc.sync.dma_start(out=xta[:, 0:4 * L], in_=xta_d[:, 0:4 * L])
            nc.scalar.dma_start(out=xta[:, 4 * L:8 * L], in_=xta_d[:, 4 * L:8 * L])
            b1x4 = load("b1x4", [128, 1], F32, b1_d[:, :])
            b2h = load("b2h", [128, 1], F32, b2_d[:, :])
            wka = consts.tile([128, 8 * DPC], BF, tag="wka")
            nc.scalar.dma_start(out=wka[:], in_=wk_d[:, :])
            wqa = load("wqa", [128, 8 * DPC], BF, wq_d[:, :])
            wva = load("wva", [128, 8 * DPC], BF, wv_d[:, :])

            xT = [xta[:, mc * L:(mc + 1) * L] for mc in range(8)]
            xj = [xje[:, mc * 256:(mc + 1) * 256] for mc in range(8)]
            wc1x4 = boot[:, 0:1024]
            we1ch = boot[:, 1024:1280]
            w2t = cpk[:, 0:1024]
            o = 1024
            trimask = cpk[:, o:o + 512]; o += 512
            ones8 = cpk[:, o:o + 64]; o += 64
            sel8 = cpk[0:8, o:o + 512]
            p8 = load("p8", [128, 256], F8, p8_d[:, :])
            P0 = p8[:, 0:128]
            P1 = p8[:, 128:256]
            wk = [wka[:, mc * DPC:(mc + 1) * DPC] for mc in range(8)]
            wq = [wqa[:, mc * DPC:(mc + 1) * DPC] for mc in range(8)]
            wv = [wva[:, mc * DPC:(mc + 1) * DPC] for mc in range(8)]

            # ---------- hej4 / hcfull4 (unblock the gelu chain) ----------
            # hcfull4[r*32+c, i] = (x @ Wc1)[i, c] + b1[c]   (replicated over r)
            ps = pps.tile([128, L], F32, tag="ps")
            for mc in range(8):
                nc.tensor.matmul(ps[:], wc1x4[:, mc * 128:(mc + 1) * 128],
                                 xT[mc], start=(mc == 0), stop=(mc == 7))
            hcfull4 = consts.tile([128, L], BF, tag="hcfull4")
            nc.vector.tensor_scalar_add(hcfull4[:], ps[:], b1x4[:, 0:1])

            # hej4[u*32+c, kk]: kk 0..31 = own A rows (j = 8k+4g+u),
            # kk 32..63 = B rows as-rank-0 (j = 8k+u, k = kk),
            # kk 64..95 = B rows as-rank-1 (j = 8k+4+u, k = kk-32)
            ps = pps.tile([128, 64], F32, tag="ps")
            for u in range(4):
                for mc in range(8):
                    nc.tensor.matmul(ps[u * CD:(u + 1) * CD, :],
                                     we1ch[:, mc * CD:(mc + 1) * CD],
                                     xj[mc][:, u * 64:(u + 1) * 64],
                                     start=(mc == 0), stop=(mc == 7),
                                     tile_position=(0, u * CD))
            hej4 = consts.tile([128, 64], F32, tag="hej4")
            nc.vector.tensor_copy(hej4[:], ps[:])

            # ---------- pairwise causal-graph bias (transposed, own half) ----
            # gallA: j-groups k 0..31 (i in [0,512)); gallB: k 32..63
            # (i in [256,512)).  fd(k) = 512 - 8k.
            gallAB = consts.tile([128, XW], F8, tag="gallAB")
            gallA = gallAB[:, 0:AW]
            gallB = gallAB[:, AW:XW]

            def pairwise(bb, graw, hoff, i0):
                ks = range(bb * 8, bb * 8 + 8)
                fds = [L - 8 * k for k in ks]
                offs = [sum(fds[:n]) for n in range(8)]
                tot = sum(fds)
                t4 = work.tile([128, 3872], BF, tag="t4", bufs=3)
                for n, k in enumerate(ks):
                    nc.vector.tensor_scalar_add(
                        t4[:, offs[n]:offs[n] + fds[n]],
                        hcfull4[:, 8 * k:L], hej4[:, hoff + k:hoff + k + 1])
                ga = work.tile([128, 3872], BF, tag="ga", bufs=3)
                nc.scalar.activation(ga[:, 0:tot], t4[:, 0:tot], AF.Gelu)
                for n, k in enumerate(ks):
                    t = k % 32
                    gb = 32 * (t // 8)
                    nc.tensor.matmul(graw[gb:gb + 32, 8 * k - i0:],
                                     w2t[:, t * 32:(t + 1) * 32],
                                     ga[:, offs[n]:offs[n] + fds[n]],
                                     start=(t % 8 == 0), stop=(t % 8 == 7),
                                     tile_position=(0, gb))

            def gfin(graw, gall, w):
                nc.scalar.activation(gall[:, 0:w], graw[:, 0:w], AF.Tanh,
                                     scale=0.5, bias=b2h[:, 0:1])

            pgr_ctx = tc.tile_pool(name="pgr", bufs=1, space="PSUM")
            pgr = pgr_ctx.__enter__()
            with tc.high_priority():
                grawA = pgr.tile([128, AW], F32, tag="graw")
                nc.scalar.activation(grawA[:], xta[:, 0:AW], AF.Copy,
                                     scale=0.0)
                for bb in range(4):
                    pairwise(bb, grawA, 0, 0)
                gfin(grawA, gallA, AW)

            # ---------- pairwise B (own half) ----------
            with tc.high_priority():
                grawB = pgr.tile([128, BW], F32, tag="graw")
                nc.scalar.activation(grawB[:], xta[:, 0:BW], AF.Copy,
                                     scale=0.0)
                for bb in range(4, 8):
                    pairwise(bb, grawB, 0, 256)
                gfin(grawB, gallB, BW)

            # ---------- exchange both halves within the pair ----------
            gin = dpool.tile([128, XW], F8, tag="gin")
            gout = dpool.tile([2, 128, XW], F8, tag="gout")
            with tc.high_priority():
                nc.sync.dma_start(out=gin[:, :], in_=gallAB[:])
                nc.gpsimd.collective_compute(
                    "AllGather", ALU.bypass,
                    replica_groups=[[0, 1], [2, 3], [4, 5], [6, 7]],
                    ins=[gin[:, :].opt()], outs=[gout[:, :, :].opt()])
            woa = load("woa", [128, 4 * D], BF, wo_d[:, :])
            wo = [woa[:, dc * D:(dc + 1) * D] for dc in range(4)]
            gx = []
            for r in range(2):
                t = consts.tile([128, XW], F8, tag=f"gx{r}")
                eng = nc.sync if r == 0 else nc.scalar
                eng.dma_start(out=t[:], in_=gout[r, :, :])
                gx.append(t)

            # ---------- projection emitters ----------
            kT, qT, v = [None] * 4, [None] * 4, [None] * 4

            def proj_kq(dc):
                ps = pps.tile([128, L], F32, tag="ps")
                for mc in range(8):
                    nc.tensor.matmul(ps[:], wk[mc][:, dc * 128:(dc + 1) * 128],
                                     xT[mc], start=(mc == 0), stop=(mc == 7))
                t = consts.tile([128, L], BF, tag=f"kT{dc}")
                nc.vector.tensor_copy(t[:], ps[:])
                kT[dc] = t
                ps = pps.tile([128, L], F32, tag="ps")
                for mc in range(8):
                    nc.tensor.matmul(ps[:], wq[mc][:, dc * 128:(dc + 1) * 128],
                                     xT[mc], start=(mc == 0), stop=(mc == 7))
                t = consts.tile([128, L], BF, tag=f"qT{dc}")
                nc.vector.tensor_copy(t[:], ps[:])
                qT[dc] = t

            def proj_v(jc):
                t = consts.tile([128, DPC], BF, tag=f"v{jc}")
                ps = pps.tile([128, DPC], F32, tag="ps")
                for mc in range(8):
                    nc.tensor.matmul(ps[:], xT[mc][:, jc * 128:(jc + 1) * 128],
                                     wv[mc], start=(mc == 0), stop=(mc == 7))
                nc.vector.tensor_copy(t[:], ps[:])
                v[jc] = t

            proj_kq(0)
            proj_kq(1)
            proj_v(2)
            proj_v(3)
            proj_kq(2)
            proj_kq(3)

            # ---------- re-interleave G^T chunks (2 perm matmuls each) ------
            # GTc[jc] covers j in [128jc, 128jc+128), i in [128jc, 512)
            attpool = ctx_att = tc.tile_pool(name="att", bufs=1, space="PSUM")
            psc = pot = prs = attpool.__enter__()

            # ---------- scores + exp(scores): independent of the exchange --
            eR = [[None] * 4 for _ in range(4)]

            def scores_exp(hp, jc):
                iext = L - 128 * jc
                sc = psc.tile([128, 1024], F32, tag="sc", bufs=2)
                for sub in range(2):
                    po = 64 * sub
                    nc.tensor.matmul(
                        sc[:, sub * 512:sub * 512 + iext],
                        kT[hp][po:po + 64, jc * 128:(jc + 1) * 128],
                        qT[hp][po:po + 64, jc * 128:L],
                        start=True, stop=True, tile_position=(po, 0))
                e = etp.tile([128, 2 * iext], BF, tag="eRaw", bufs=16)
                src3 = sc[:].rearrange("p (s n) -> p s n", s=2)[:, :, 0:iext]
                dst3 = e[:].rearrange("p (s n) -> p s n", s=2)[:, :, 0:iext]
                nc.scalar.activation(dst3, src3, AF.Exp)
                eR[hp][jc] = e

            for hp in range(4):
                for jc in (2, 3, 0, 1):
                    scores_exp(hp, jc)

            # late-emitted so the scheduler parks these in the PE hole that
            # opens while the collective drains (their consumers run after)
            proj_v(0)
            proj_v(1)

            GTc = [None] * 4

            def make_gtc(jc):
                iext = L - 128 * jc
                half = slice(0, 64) if jc % 2 == 0 else slice(64, 128)
                src = ([gx[0][:, 0:AW], gx[1][:, 0:AW]] if jc < 2
                       else [gx[0][:, AW:XW], gx[1][:, AW:XW]])
                cs = slice(128 * (jc % 2), (AW if jc < 2 else BW))
                ps = pps.tile([128, 512], F32, tag="ps")
                nc.tensor.matmul(ps[:, 0:iext], P0[half, :], src[0][half, cs],
                                 start=True, stop=False)
                nc.tensor.matmul(ps[:, 0:iext], P1[half, :], src[1][half, cs],
                                 start=False, stop=True)
                t = consts.tile([128, iext], BF, tag=f"GTc{jc}")
                nc.vector.scalar_tensor_tensor(
                    t[:], ps[:, 0:iext], ALPHA / 2.0, trimask[:, 0:iext],
                    op0=ALU.mult, op1=ALU.add)
                eg = consts.tile([128, iext], BF, tag=f"expG{jc}")
                nc.scalar.activation(eg[:], t[:], AF.Exp)
                GTc[jc] = eg

            # ---------- attention ----------
            # pass A: scores + exp for everything (independent of the
            # exchange), e2 and row sums for the locally-available jc 2/3.
            rs = prs.tile([128, 512], F32, tag="rs")
            ot = [None] * 4
            eT = [[None] * 4 for _ in range(4)]

            def rowsum(hp, jc, start_ok):
                # every MM writes partitions 0..7 (row h = sum, others +0) so
                # a single accumulation group spans all heads
                iext = L - 128 * jc
                for sub in range(2):
                    h = 2 * hp + sub
                    first = start_ok and jc == 2 and sub == 0
                    last = hp == 3 and jc == 1 and sub == 1
                    if jc == 0:
                        # split at the 256-col boundary: left half is fresh
                        # (overwrite-by-bit), right half accumulates
                        for lo, hi in ((0, 256), (256, 512)):
                            nc.tensor.matmul(
                                rs[0:8, lo:hi],
                                ones8[:, 8 * h:8 * h + 8],
                                eT[hp][jc][:, sub * iext + lo:sub * iext + hi],
                                start=False, stop=False)
                    else:
                        nc.tensor.matmul(
                            rs[0:8, 128 * jc:L],
                            ones8[:, 8 * h:8 * h + 8],
                            eT[hp][jc][:, sub * iext:(sub + 1) * iext],
                            start=first, stop=last)

            for jc in (2, 3, 0, 1):
                make_gtc(jc)

            for hp in range(4):
                for jc in (2, 3, 0, 1):
                    iext = L - 128 * jc
                    e2 = etp.tile([128, 2 * iext], BF, tag="eT", bufs=16)
                    for sub in range(2):
                        nc.vector.scalar_tensor_tensor(
                            e2[:, sub * iext:(sub + 1) * iext],
                            eR[hp][jc][:, sub * iext:(sub + 1) * iext], 1.0,
                            GTc[jc][:], op0=ALU.mult, op1=ALU.mult)
                    eT[hp][jc] = e2
                for jc in (2, 3, 0, 1):
                    rowsum(hp, jc, start_ok=(hp == 0))

            inv8 = work.tile([8, 512], BF, tag="inv8")
            with nc.allow_low_precision(reason="bf16 softmax inv scale"):
                nc.vector.reciprocal(inv8[:], rs[0:8, :])

            def attention(hp):
                otp = psc.tile([128, 512], F32, tag="sc", bufs=2)
                for sub in range(2):
                    h = 2 * hp + sub
                    po = 64 * sub
                    for jc in (2, 3, 0, 1):
                        iext = L - 128 * jc
                        if jc == 0:
                            for lo, hi in ((0, 256), (256, 512)):
                                nc.tensor.matmul(
                                    otp[po:po + 64, lo:hi],
                                    v[jc][:, h * HD:(h + 1) * HD],
                                    eT[hp][jc][:, sub * iext + lo:sub * iext + hi],
                                    start=False, stop=False,
                                    tile_position=(0, po))
                        else:
                            nc.tensor.matmul(
                                otp[po:po + 64, 128 * jc:L],
                                v[jc][:, h * HD:(h + 1) * HD],
                                eT[hp][jc][:, sub * iext:(sub + 1) * iext],
                                start=(jc == 2), stop=(jc == 1),
                                tile_position=(0, po))
                ips = pps.tile([128, 512], F32, tag="ps")
                nc.tensor.matmul(ips[:], sel8[:, hp * 128:(hp + 1) * 128],
                                 inv8[:], start=True, stop=True)
                otraw = work.tile([128, 512], BF, tag="otraw")
                nc.vector.tensor_copy(otraw[:], otp[:])
                t = consts.tile([128, 512], BF, tag=f"ot{hp}")
                nc.vector.scalar_tensor_tensor(t[:], otraw[:], 1.0, ips[:],
                                               op0=ALU.mult, op1=ALU.mult)
                ot[hp] = t

            for hp in range(4):
                attention(hp)

            # ---------- output projection ----------
            def out_proj(ic, nn):
                ps = pps.tile([128, 512], F32, tag="ps")
                for dc in range(4):
                    nc.tensor.matmul(ps[:], ot[dc][:, ic * 128:(ic + 1) * 128],
                                     wo[dc][:, nn * 512:(nn + 1) * 512],
                                     start=(dc == 0), stop=(dc == 3))
                osb = work.tile([128, 512], BF, tag="osb")
                nc.vector.tensor_copy(osb[:], ps[:])
                nc.sync.dma_start(
                    out=out_d[ic * 128:(ic + 1) * 128, nn * 512:(nn + 1) * 512],
                    in_=osb[:])

            for ic in range(4):
                for nn in range(2):
                    out_proj(ic, nn)

            ctx_att.__exit__(None, None, None)
            pgr_ctx.__exit__(None, None, None)

    nc.compile()
    return nc


def _host_inputs(x, Wq, Wk, Wv, Wo, Wc, We, W1c, W1e, b1, W2, b2):
    """Per-core input dicts (host-side shard/cast/pack)."""
    x = _f32(np.asarray(x))
    wq_s = _f32(np.asarray(Wq) / math.sqrt(HD))
    wk = _f32(np.asarray(Wk))
    wv = _f32(np.asarray(Wv))
    wo = _f32(np.asarray(Wo))
    wc1 = _f32(np.asarray(Wc) @ np.asarray(W1c))      # (D, CD)
    we1 = _f32(np.asarray(We) @ np.asarray(W1e))
    wc1c = wc1.reshape(8, 128, CD).transpose(1, 0, 2)          # (128, 8, CD)
    wc1x4 = np.tile(wc1c[:, :, None, :], (1, 1, 4, 1)).reshape(128, 8 * 128)
    we1r = we1.reshape(8, 128, CD).transpose(1, 0, 2).reshape(128, 8 * CD)
    b1x4 = _f32(np.tile(np.asarray(b1).reshape(1, CD), (4, 1)).reshape(128, 1))
    b2h = _f32(np.full((128, 1), 0.5 * float(np.asarray(b2).reshape(-1)[0])))
    w2 = _f32(np.asarray(W2))

    # w2t[p=u*32+c, t*32 + m'] = W2[c] if m' == 4*(t%8)+u else 0
    # (out partition group 32*(t//8) selected by tile_position at emit time)
    w2t = np.zeros((32, 128, 32), np.float32)
    for t in range(32):
        for u in range(4):
            w2t[t, u * CD:(u + 1) * CD, 4 * (t % 8) + u] = w2
    w2t = w2t.transpose(1, 0, 2).reshape(128, 32 * 32)

    identb = np.eye(128, dtype=np.float32)
    # P_r[4k+u, (8k + 4r + u) % 128] = 1  (fp8 operand for the gx permutes)
    P = np.zeros((2, 128, 128), np.float32)
    for r in range(2):
        for k in range(32):
            for u in range(4):
                P[r, 4 * k + u, (8 * k + 4 * r + u) % 128] = 1.0
    # trimask[m, x] = 0 iff x >= m (i >= j within any j-chunk), else -inf
    trimask = np.where(np.arange(512)[None, :] >= np.arange(128)[:, None],
                       0.0, NEG).astype(np.float32)
    ones8 = np.zeros((128, 64), np.float32)
    for h in range(8):
        ones8[:, 8 * h + h] = 1.0
    sel8 = np.zeros((128, 512), np.float32)
    for hp in range(4):
        sel8[2 * hp, hp * 128:hp * 128 + 64] = 1.0
        sel8[2 * hp + 1, hp * 128 + 64:hp * 128 + 128] = 1.0
    bootc = np.concatenate([wc1x4, we1r], axis=1)

    def hpack(w, cols):  # (1024, cols) -> (128, 8*cols) m-chunk-major
        return w.reshape(8, 128, cols).transpose(1, 0, 2).reshape(128, 8 * cols)

    in_maps = []
    for core in range(N_CORES):
        b, g = core // 2, core % 2
        hd0 = g * DPC                                  # head-group d offset
        xTb = np.ascontiguousarray(x[b].T)             # (D, L)
        # hej columns: own A rows (j = 8k+4g+u, k<32), then B rows for both
        # rank parities (j = 8k+4r+u, k in 32..64, r = 0 then 1)
        jcols = np.array([8 * kk + 4 * g + u
                          for u in range(4) for kk in range(64)])
        xjeb = np.ascontiguousarray(xTb[:, jcols])     # (D, 384)
        cpk = np.concatenate([w2t, trimask, ones8, sel8], axis=1)
        assert cpk.shape[1] == CPK
        in_maps.append({
            "boot": _bf(bootc),
            "xta": _bf(hpack(xTb, L)),
            "xje": _bf(hpack(xjeb, 256)),
            "cpk": _bf(cpk),
            "p8": np.ascontiguousarray(
                np.concatenate([P[0], P[1]], axis=1)
                .astype(ml_dtypes.float8_e4m3)),
            "wka": _bf(hpack(wk[:, hd0:hd0 + DPC], DPC)),
            "wqa": _bf(hpack(wq_s[:, hd0:hd0 + DPC], DPC)),
            "wva": _bf(hpack(wv[:, hd0:hd0 + DPC], DPC)),
            "woa": _bf(np.ascontiguousarray(
                wo[hd0:hd0 + DPC].reshape(4, 128, D)
                .transpose(1, 0, 2).reshape(128, 4 * D))),
            "b1x4": b1x4, "b2h": b2h,
        })
    return in_maps


def run(inputs: dict, trace: bool = False):
    """Build, run on 8 cores, return (full_output, BassKernelResults)."""
    nc = build_program()
    in_maps = _host_inputs(**inputs)
    res = run_bass_kernel_spmd(nc, in_maps, core_ids=list(range(N_CORES)),
                               trace=trace)
    out = np.zeros((B, L, D), np.float32)
    for b in range(B):
        out[b] = (res.results[2 * b]["out"].astype(np.float32)
                  + res.results[2 * b + 1]["out"].astype(np.float32))
    return out, res


def kernel(**inputs) -> np.ndarray:
    out, _ = run(inputs, trace=False)
    return out

